# revision 25
# baseline (speedup 1.0000x reference)
"""Trainium2 Bass kernel for LoRA multi-head attention (B=2, S=2048, D=768, H=12, R=8).

Sharding over 8 cores: (batch, query-half, head-half) -> each core computes
6 heads x 1024 query rows x full 2048 keys, producing a partial (over the
head dimension) of the final merge projection. Host sums the two head-half
partials per (batch, query-half) slice.

All activations are kept feature-major ("transposed") on device so every
matmul contraction lands on the partition axis with no on-device transposes.
Matmuls run in fp32r (full PE rate); attention probabilities in bf16.
"""

import sys

if "/opt/trn_rl_repo" not in sys.path:
    sys.path.insert(0, "/opt/trn_rl_repo")

import numpy as np

import concourse.bass as bass
import concourse.tile as tile
from concourse import bacc, mybir
from concourse.bass_utils import run_bass_kernel_spmd

F32 = mybir.dt.float32
F32R = mybir.dt.float32r
BF16 = mybir.dt.bfloat16
I32 = mybir.dt.int32
EXP = mybir.ActivationFunctionType.Exp

B, S, D, H, R = 2, 2048, 768, 12, 8
DK = D // H  # 64
NCORES = 8
HPC = 6            # heads per core
HDIM = HPC * DK    # 384: head-slice width per core
QR = S // 2        # 1024 query rows per core
SC = 512           # streaming chunk (s dimension)
NSC = S // SC      # 4
NQC = QR // SC     # 2 query chunks per core
NKT = S // 128     # 16 key tiles
DO = D // 128      # 6 d-chunks

_CACHE = {}


def _build_kernel():
    """Build the full Bass program. One SPMD program serves all 8 cores; the
    (batch, q-half, head-half) selection is done host-side via input slicing.

    Emission order is software-pipelined so the scalar engine's exp chain
    starts right after the k/q projections, overlapping the v projection:
      pass1: k-proj, BW, uv, q-proj
      scores(q0,p0), scores(q1,p0)
      pass2: uq + v-proj
      attv/scores interleaved tail, merges
    """
    nc = bacc.Bacc("TRN2", target_bir_lowering=False, debug=False,
                   enable_asserts=True, num_devices=NCORES)

    def din(name, shape, dt=BF16):
        return nc.dram_tensor(name, shape, dt, kind="ExternalInput").ap()

    qT_d = din("qT", [D, S])
    kT_d = din("kT", [D, S])
    vTh_d = din("vTh", [D, QR])
    maskT_d = din("maskT", [S, QR])
    wqt_d = din("wqt", [D, HDIM])
    wkt_d = din("wkt", [D, HDIM])
    wvt_d = din("wvt", [D, HDIM])
    wmt_d = din("wmt", [HDIM, D])
    bqc_d = din("bqc", [128, 3], F32)
    bkc_d = din("bkc", [128, 3], F32)
    bvr_d = din("bvr", [1, HDIM])
    bmr_d = din("bmr", [1, D])
    aq_d = din("aq", [D, R])
    av_d = din("av", [D, R])
    bqt_d = din("bqt", [D, R])
    bvt_d = din("bvt", [D, R])
    out_d = nc.dram_tensor("out", [QR, D], F32, kind="ExternalOutput").ap()

    with tile.TileContext(nc) as tc:
        with (
            tc.tile_pool(name="keep", bufs=1) as keep,
            tc.tile_pool(name="maskp", bufs=2) as maskp,
            tc.tile_pool(name="wa", bufs=1) as wa,
            tc.tile_pool(name="acts", bufs=2) as acts,
            tc.tile_pool(name="bpool", bufs=2) as bpool,
            tc.tile_pool(name="epool", bufs=2) as epool,
            tc.tile_pool(name="npool", bufs=2) as npool,
            tc.tile_pool(name="fpool", bufs=2) as fpool,
            tc.tile_pool(name="dpool", bufs=4, space="DRAM") as dpool,
            tc.tile_pool(name="psproj", bufs=2, space="PSUM") as psproj,
            tc.tile_pool(name="pss", bufs=2, space="PSUM") as pss,
            tc.tile_pool(name="pso", bufs=2, space="PSUM") as pso,
        ):
            # ---- persistent tiles ----
            wm_sb = keep.tile([128, 3, D], BF16)
            ones_sb = keep.tile([1, 128], BF16)
            nc.vector.memset(ones_sb[:], 1.0)
            bm_sb = keep.tile([1, D], BF16)
            kT_sb = keep.tile([128, 3, S], BF16)
            qT_sb = keep.tile([128, 3, QR], BF16)
            v_aug = keep.tile([128, NKT, HPC, DK + 1], BF16)
            nc.vector.memset(v_aug[:, :, :, DK:DK + 1], 1.0)

            def wload(name, dram, shape, pat):
                t = wa.tile(shape, BF16, name=name)
                nc.sync.dma_start(t[:], dram.rearrange(pat, p=128))
                return t

            wk_sb = wa.tile([128, DO, HDIM], BF16, name="wk_sb")
            for _et in range(3):
                _esl = slice(_et * 128, (_et + 1) * 128)
                nc.sync.dma_start(
                    wk_sb[:, :, _esl],
                    wkt_d.rearrange("(o p) e -> p o e", p=128)[:, :, _esl])
            kTc0 = acts.tile([128, DO, SC], BF16, tag="act", name="kTc0")
            nc.sync.dma_start(
                kTc0[:, 0:3, :],
                kT_d.rearrange("(o p) s -> p o s", p=128)[:, 0:3, 0:SC])
            nc.sync.dma_start(
                kTc0[:, 3:6, :],
                kT_d.rearrange("(o p) s -> p o s", p=128)[:, 3:6, 0:SC])
            wq_sb = wload("wq_sb", wqt_d, [128, DO, HDIM], "(o p) e -> p o e")
            wv_sb = wload("wv_sb", wvt_d, [128, DO, HDIM], "(o p) e -> p o e")
            bqt_sb = wload("bqt_sb", bqt_d, [128, DO, R], "(o p) r -> p o r")
            bvt_sb = wload("bvt_sb", bvt_d, [128, DO, R], "(o p) r -> p o r")
            aq_sb = wload("aq_sb", aq_d, [128, DO, R], "(o p) r -> p o r")
            av_sb = wload("av_sb", av_d, [128, DO, R], "(o p) r -> p o r")
            bq_sb = wa.tile([128, 3], F32)
            nc.sync.dma_start(bq_sb[:], bqc_d[:])
            bk_sb = wa.tile([128, 3], F32)
            nc.sync.dma_start(bk_sb[:], bkc_d[:])
            bv_sb = wa.tile([1, HDIM], BF16)
            nc.sync.dma_start(bv_sb[:], bvr_d[:])
            uq_sb = wa.tile([R, S], BF16)
            uv_sb = wa.tile([R, QR], BF16)
            BWq_sb = wa.tile([R, HDIM], BF16)
            BWv_sb = wa.tile([R, HDIM], BF16)

            # ---- pass 1: k-projection (T-form), BW, uv, q-projection ----
            for sc in range(NSC):
                ssl = slice(sc * SC, (sc + 1) * SC)
                if sc == 0:
                    kTc = kTc0
                else:
                    kTc = acts.tile([128, DO, SC], BF16, tag="act", name="kTc")
                    nc.sync.dma_start(
                        kTc[:],
                        kT_d.rearrange("(o p) s -> p o s", p=128)[:, :, ssl])
                for et in range(3):
                    esl = slice(et * 128, (et + 1) * 128)
                    ps = psproj.tile([128, 512], F32, tag="pp", name="ps_k")
                    for do in range(DO):
                        nc.tensor.matmul(ps[:], wk_sb[:, do, esl], kTc[:, do, :],
                                         start=(do == 0), stop=(do == DO - 1))
                    nc.vector.tensor_scalar_add(
                        kT_sb[:, et, ssl], ps[:], bk_sb[:, et:et + 1])
                if sc == 0:
                    ps = psproj.tile([128, 512], F32, tag="pp", name="ps_bwq")
                    for do in range(DO):
                        nc.tensor.matmul(ps[:R, :HDIM], bvt_sb[:, do, :],
                                         wq_sb[:, do, :],
                                         start=(do == 0), stop=(do == DO - 1))
                    nc.vector.tensor_copy(BWq_sb[:], ps[:R, :HDIM])
                    ps = psproj.tile([128, 512], F32, tag="pp", name="ps_bwv")
                    for do in range(DO):
                        nc.tensor.matmul(ps[:R, :HDIM], bqt_sb[:, do, :],
                                         wv_sb[:, do, :],
                                         start=(do == 0), stop=(do == DO - 1))
                    nc.vector.tensor_copy(BWv_sb[:], ps[:R, :HDIM])

            for qc in range(NQC):
                qsl = slice(qc * SC, (qc + 1) * SC)
                vThc = acts.tile([128, DO, SC], BF16, tag="act2", name="vThc")
                nc.sync.dma_start(
                    vThc[:], vTh_d.rearrange("(o p) s -> p o s", p=128)[:, :, qsl])
                ps = psproj.tile([128, 512], F32, tag="pp", name="ps_uv")
                for do in range(DO):
                    nc.tensor.matmul(ps[:R, :], av_sb[:, do, :], vThc[:, do, :],
                                     start=(do == 0), stop=(do == DO - 1))
                nc.vector.tensor_copy(uv_sb[:, qsl], ps[:R, :])
                for et in range(3):
                    esl = slice(et * 128, (et + 1) * 128)
                    ps = psproj.tile([128, 512], F32, tag="pp", name="ps_q")
                    for do in range(DO):
                        nc.tensor.matmul(ps[:], wq_sb[:, do, esl], vThc[:, do, :],
                                         start=(do == 0), stop=False)
                    nc.tensor.matmul(ps[:], BWq_sb[:, esl], uv_sb[:, qsl],
                                     start=False, stop=True)
                    nc.vector.tensor_scalar_add(
                        qT_sb[:, et, qsl], ps[:], bq_sb[:, et:et + 1])

            # ---- phase B helpers ----
            m01s = {}
            atts = {}

            def emit_scores_pair(qc, p):
                # heads 2p (rows 0-63) and 2p+1 (rows 64-127) interleave as PE
                # row-groups and run concurrently; one [128,2,SC] psum tile
                # holds the same key-tile for both heads so exp and the mask
                # multiply process both heads in single ops.
                qsl = slice(qc * SC, (qc + 1) * SC)
                if qc not in m01s:
                    m01 = maskp.tile([128, NKT, SC], BF16, tag="mb", name="m01")
                    nc.sync.dma_start(
                        m01[:],
                        maskT_d.rearrange("(o p) q -> p o q", p=128)[:, :, qsl])
                    m01s[qc] = m01
                m01 = m01s[qc]
                att = bpool.tile([128, NKT, 2, SC], BF16, tag="att", name="att")
                for kt in range(NKT):
                    ps_s = pss.tile([128, 2, SC], F32, tag="ss", name="ps_s")
                    for j in range(2):
                        po = j * 64
                        nc.tensor.matmul(
                            ps_s[:, j, :],
                            kT_sb[po:po + 64, p, kt * 128:(kt + 1) * 128],
                            qT_sb[po:po + 64, p, qsl],
                            start=True, stop=True)
                    exp_t = epool.tile([128, 2, SC], BF16, tag="ex", name="exp_t")
                    nc.scalar.activation(exp_t[:], ps_s[:], EXP, scale=0.125)
                    mk = m01[:, kt, None, :].to_broadcast([128, 2, SC])
                    nc.vector.tensor_mul(att[:, kt, :, :], exp_t[:], mk)
                atts[(qc, p)] = att

            def emit_attv_pair(qc, p):
                att = atts.pop((qc, p))
                for j in range(2):
                    h = 2 * p + j
                    po = j * 64
                    ps_o = pso.tile([DK + 1, SC], F32, tag="oo", name="ps_o")
                    for kt in range(NKT):
                        nc.tensor.matmul(ps_o[:], v_aug[:, kt, h, :],
                                         att[:, kt, j, :],
                                         start=(kt == 0), stop=(kt == NKT - 1))
                    ob = npool.tile([DK, SC], F32, tag="ob", name="ob")
                    nc.vector.tensor_copy(ob[:], ps_o[0:DK, :])
                    den_sb = npool.tile([1, SC], F32, tag="den", name="den_sb")
                    nc.vector.tensor_copy(den_sb[:], ps_o[DK:DK + 1, :])
                    r_sb = npool.tile([1, SC], F32, tag="r", name="r_sb")
                    nc.vector.reciprocal_approx_fast(r_sb[:], den_sb[:])
                    dr = dpool.tile([1, SC], F32, tag="dr", name="dr")
                    nc.sync.dma_start(dr[:], r_sb[:])
                    bb = npool.tile([64, SC], F32, tag="bb", name="bb")
                    dr_ap = dr[:]
                    bcast = bass.AP(tensor=dr_ap.tensor, offset=dr_ap.offset,
                                    ap=[[0, 64]] + list(dr_ap.ap)[1:])
                    nc.gpsimd.dma_start(bb[:], bcast)
                    if h == 0:
                        _OUTT[qc] = bpool.tile([128, 3, SC], BF16, tag="outT",
                                               name="outT")
                    outT_sb = _OUTT[qc]
                    nc.vector.tensor_mul(outT_sb[po:po + 64, p, :],
                                         ob[0:DK, :], bb[:])

            def emit_merge(qc):
                outT_sb = _OUTT[qc]
                for qt in range(4):
                    qtsl = slice(qt * 128, (qt + 1) * 128)
                    for ec in range(2):
                        esl = slice(ec * 384, (ec + 1) * 384)
                        ps_m = psproj.tile([128, 512], F32, tag="pp", name="ps_m")
                        for hp in range(3):
                            nc.tensor.matmul(ps_m[:, :384], outT_sb[:, hp, qtsl],
                                             wm_sb[:, hp, esl],
                                             start=(hp == 0), stop=False)
                        nc.tensor.matmul(ps_m[:, :384], ones_sb[:], bm_sb[:, esl],
                                         start=False, stop=True)
                        fin = fpool.tile([128, 384], F32, tag="fin", name="fin")
                        nc.scalar.copy(fin[:], ps_m[:, :384])
                        nc.sync.dma_start(
                            out_d[qc * SC + qt * 128:qc * SC + (qt + 1) * 128, esl],
                            fin[:])

            # ---- pass 2: uq + v-projection (natural layout), interleaved
            # with the first scores pairs so ACT/DVE fill early without the
            # projection copybacks queueing behind the mask multiplies ----
            def emit_pass2(sc):
                ssl = slice(sc * SC, (sc + 1) * SC)
                qTc = acts.tile([128, DO, SC], BF16, tag="act2", name="qTc")
                nc.sync.dma_start(
                    qTc[:], qT_d.rearrange("(o p) s -> p o s", p=128)[:, :, ssl])
                ps = psproj.tile([128, 512], F32, tag="pp", name="ps_uq")
                for do in range(DO):
                    nc.tensor.matmul(ps[:R, :], aq_sb[:, do, :], qTc[:, do, :],
                                     start=(do == 0), stop=(do == DO - 1))
                nc.vector.tensor_copy(uq_sb[:, ssl], ps[:R, :])
                for st in range(4):
                    gst = sc * 4 + st
                    stsl = slice(st * 128, (st + 1) * 128)
                    ps = psproj.tile([128, 512], F32, tag="pp", name="ps_v")
                    for do in range(DO):
                        nc.tensor.matmul(ps[:, :HDIM], qTc[:, do, stsl],
                                         wv_sb[:, do, :],
                                         start=(do == 0), stop=False)
                    nc.tensor.matmul(ps[:, :HDIM],
                                     uq_sb[:, gst * 128:(gst + 1) * 128], BWv_sb[:],
                                     start=False, stop=False)
                    nc.tensor.matmul(ps[:, :HDIM], ones_sb[:], bv_sb[:],
                                     start=False, stop=True)
                    nc.vector.tensor_copy(
                        v_aug[:, gst, :, 0:DK],
                        ps[:, :HDIM].rearrange("p (h d) -> p h d", h=HPC))

            emit_pass2(0)
            emit_scores_pair(0, 0)
            emit_pass2(1)
            emit_scores_pair(1, 0)
            emit_pass2(2)
            emit_pass2(3)

            # merge weights needed only at the end
            nc.sync.dma_start(wm_sb[:], wmt_d.rearrange("(o p) e -> p o e", p=128))
            nc.sync.dma_start(bm_sb[:], bmr_d[:])

            # ---- pipelined attention tail ----
            emit_attv_pair(0, 0)
            emit_scores_pair(0, 1)
            emit_attv_pair(1, 0)
            emit_scores_pair(1, 1)
            emit_attv_pair(0, 1)
            emit_scores_pair(0, 2)
            emit_attv_pair(1, 1)
            emit_scores_pair(1, 2)
            emit_attv_pair(0, 2)
            emit_merge(0)
            emit_attv_pair(1, 2)
            emit_merge(1)

    nc.compile()
    return nc


_OUTT = {}


def _shard_inputs(inputs):
    q = np.asarray(inputs["query"], np.float32)
    k = np.asarray(inputs["key"], np.float32)
    v = np.asarray(inputs["value"], np.float32)
    mask = np.asarray(inputs["mask"], np.int32)
    Wq = np.asarray(inputs["Wq"], np.float32)
    Wk = np.asarray(inputs["Wk"], np.float32)
    Wv = np.asarray(inputs["Wv"], np.float32)
    Wm = np.asarray(inputs["Wm"], np.float32)
    bq = np.asarray(inputs["bq"], np.float32)
    bk = np.asarray(inputs["bk"], np.float32)
    bv = np.asarray(inputs["bv"], np.float32)
    bm = np.asarray(inputs["bm"], np.float32)
    Aq = np.asarray(inputs["lora_A_q"], np.float32)
    Bq = np.asarray(inputs["lora_B_q"], np.float32)
    Av = np.asarray(inputs["lora_A_v"], np.float32)
    Bv = np.asarray(inputs["lora_B_v"], np.float32)

    import ml_dtypes
    bf16 = ml_dtypes.bfloat16

    def c(x):
        return np.ascontiguousarray(x)

    def cb(x):
        return np.ascontiguousarray(x.astype(bf16))

    qT = [cb(q[b].T) for b in range(B)]
    kT = [cb(k[b].T) for b in range(B)]
    vT = [cb(v[b].T) for b in range(B)]
    mT = [cb(mask[b].T) for b in range(B)]
    WqT, WkT, WvT, WmT = cb(Wq.T), cb(Wk.T), cb(Wv.T), cb(Wm.T)
    BqT, BvT = cb(Bq.T), cb(Bv.T)

    in_maps = []
    for core in range(NCORES):
        b, qh, hh = core // 4, (core // 2) % 2, core % 2
        hsl = slice(hh * HDIM, (hh + 1) * HDIM)
        qrows = slice(qh * QR, (qh + 1) * QR)
        in_maps.append({
            "qT": qT[b],
            "kT": kT[b],
            "vTh": c(vT[b][:, qrows]),
            "maskT": c(mT[b][:, qrows]),
            "wqt": c(WqT[:, hsl]),
            "wkt": c(WkT[:, hsl]),
            "wvt": c(WvT[:, hsl]),
            "wmt": c(WmT[hsl, :]),
            "bqc": c(bq[hsl].reshape(3, 128).T),
            "bkc": c(bk[hsl].reshape(3, 128).T),
            "bvr": cb(bv[hsl].reshape(1, HDIM)),
            "bmr": cb((bm if hh == 0 else np.zeros_like(bm)).reshape(1, D)),
            "aq": cb(Aq), "av": cb(Av), "bqt": BqT, "bvt": BvT,
        })
    return in_maps


def _get_nc():
    if "nc" not in _CACHE:
        _CACHE["nc"] = _build_kernel()
    return _CACHE["nc"]


def kernel(**inputs) -> np.ndarray:
    nc = _get_nc()
    in_maps = _shard_inputs(inputs)
    res = run_bass_kernel_spmd(nc, in_maps, core_ids=list(range(NCORES)))
    out = np.zeros((B, S, D), np.float32)
    for b in range(B):
        for qh in range(2):
            part = (res.results[b * 4 + qh * 2 + 0]["out"]
                    + res.results[b * 4 + qh * 2 + 1]["out"])
            out[b, qh * QR:(qh + 1) * QR, :] = part
    return out
